# revision 1
# baseline (speedup 1.0000x reference)
"""Trainium2 Bass kernel for nn_MetaGraphLearner (GNN edge scorer).

Math (reference):
  t  = X @ Wt.T + bt                  [B,N,H]
  hi = t @ W1i.T, hj = t @ W1j.T      [B,N,E]   (W1 = [W1i | W1j])
  ew[b,i,j] = sum_e W2[e]*relu(hi[b,i,e]+hj[b,j,e]+b1[e]) + b2
  out = sigmoid(ew) * adj[None]

Since hi/hj are linear in X, Wt is folded into W1 host-side (weight algebra
only): hj[j,e] = X[j] @ (W1j@Wt)[e] + (W1j@bt)[e], same for hi. All constants
(b1, W1j@bt, W1i@bt) fold into one per-(k,e) bias vector.

Kernel (per core, 8 cores, each owns 256 of the B*N=2048 rows):
  Partition layout p = 32*k + e  (k in 0..3 row-sublane, e in 0..31).
  hjT4[p, j]  = |W2[e]|*(hj[j,e] + allbias[e])  one [128,1024] bf16 tile:
                2 matmuls from X.T with combined weights + bias on the copy
  Hb[p, g]    = |W2[e]| * hi_lin[4g+k, e]       [128,64] via 4 matmuls
  R_g[p, j]   = max(hjT4[p,j] + Hb[p,g], 0)     = |W2[e]|*relu(hi+hj+b1)
                one ScalarE activation (relu w/ per-partition bias) or one
                VectorE tensor_scalar (add,max) per group of 4 output rows
  psum[4g+k,j] += sign(W2[e]) * R_g[p,j]        TensorE bf16 matmul, sliding
                [128,128] window of a constant sign strip, 32 groups
                accumulate into one PSUM bank
  out = sigmoid(psum + b2) * adj                ScalarE sigmoid + VectorE mul

Raw Bass (explicit engine blocks + semaphores): the walrus build here rejects
any instruction carrying >1 attached sync wait, so all cross-engine waits are
standalone wait_ge instructions, one semaphore each.
"""

import sys

if "/opt/trn_rl_repo" not in sys.path:
    sys.path.insert(0, "/opt/trn_rl_repo")

import numpy as np
from contextlib import ExitStack

B, N, H, E = 2, 1024, 128, 32
NCORES = 8
ROWS_PER_CORE = (B * N) // NCORES  # 256
NSLAB = 2                 # slabs of 128 rows per core
GPS = 32                  # groups (of 4 rows) per slab
NG = NSLAB * GPS          # 64 groups per core
RBUF = 8                  # R-tile ring slots
NSCA = 14                 # groups handled by ScalarE (rest on VectorE)

# bf16 input "xtb" free-axis layout: [ X.T (N) | X2.T (256) ]
XTB_W = N + ROWS_PER_CORE
# bf16 input "wb" free-axis layout: [ sgn(256) | wc(128) | wci(32) ]
SGN_O, WC_O, WCI_O = 0, 256, 384
WB_W = 416
# f32 input "cin": [ bvec(1) | b2c(1) ]
CIN_W = 2

SCA_GG = [g for g in range(NG) if (g * NSCA) // NG != ((g + 1) * NSCA) // NG]
VEC_GG = [g for g in range(NG) if g not in SCA_GG]

_CACHE = {}


def _build_program():
    import concourse.bass as bass
    import concourse.mybir as mybir

    f32 = mybir.dt.float32
    bf16 = mybir.dt.bfloat16
    AF = mybir.ActivationFunctionType
    ALU = mybir.AluOpType

    nc = bass.Bass()
    xtb = nc.declare_dram_parameter("xtb", [128, XTB_W], bf16, isOutput=False)
    wb = nc.declare_dram_parameter("wb", [128, WB_W], bf16, isOutput=False)
    cin = nc.declare_dram_parameter("cin", [128, CIN_W], f32, isOutput=False)
    adj = nc.declare_dram_parameter("adj", [ROWS_PER_CORE, N], f32, isOutput=False)
    out_d = nc.declare_dram_parameter("out", [ROWS_PER_CORE, N], f32, isOutput=True)

    # producer-ordinal bookkeeping for the R ring
    vcnt, scnt = {}, {}
    v = s = 0
    for g in range(NG):
        if g in SCA_GG:
            s += 1
            scnt[g] = s
        else:
            v += 1
            vcnt[g] = v
    N_PREP_MM = 6  # 2 hj + 4 hb

    with ExitStack() as ctx:
        EN = ctx.enter_context
        xtb_sb = EN(nc.sbuf_tensor("xtb_sb", [128, XTB_W], bf16))
        wb_sb = EN(nc.sbuf_tensor("wb_sb", [128, WB_W], bf16))
        cin_sb = EN(nc.sbuf_tensor("cin_sb", [128, CIN_W], f32))
        warm_sb = EN(nc.sbuf_tensor("warm_sb", [128, 1], f32))
        wmm_sb = EN(nc.sbuf_tensor("wmm_sb", [128, 4], bf16))
        hj_sb = EN(nc.sbuf_tensor("hj_sb", [128, N], bf16))
        hb_sb = EN(nc.sbuf_tensor("hb_sb", [128, NG], f32))
        r_sb = [EN(nc.sbuf_tensor(f"r{i}", [128, N], bf16)) for i in range(RBUF)]
        adj_sb = [EN(nc.sbuf_tensor(f"adj{s_}", [128, N], f32)) for s_ in range(NSLAB)]
        o_sb = [EN(nc.sbuf_tensor(f"o{s_}", [128, N], f32)) for s_ in range(NSLAB)]

        # PSUM: hj needs 2 banks, hb 1, main acc 4
        hjp = [EN(nc.psum_tensor(f"hjp{h}", [128, 512], f32)) for h in range(2)]
        hb_ps = EN(nc.psum_tensor("hbp", [128, 512], f32))
        acc_ps = [EN(nc.psum_tensor(f"acc{i}", [128, 512], f32)) for i in range(4)]

        sem_xta = EN(nc.semaphore("sxta"))
        sem_xtb = EN(nc.semaphore("sxtb"))
        sem_xt2 = EN(nc.semaphore("sxt2"))
        sem_wb = EN(nc.semaphore("swb"))
        sem_cin = EN(nc.semaphore("scin"))
        sem_adj = EN(nc.semaphore("sadj"))
        sem_out = EN(nc.semaphore("sout"))
        sem_pe = EN(nc.semaphore("spe"))
        sem_act = EN(nc.semaphore("sact"))
        sem_vec = EN(nc.semaphore("svec"))
        sem_vR = EN(nc.semaphore("svr"))
        sem_warm = EN(nc.semaphore("swarm"))
        sem_pool = EN(nc.semaphore("spool"))
        sem_sR = EN(nc.semaphore("ssr"))

        xt_a = xtb_sb[:, 0:N]
        wc_a = wb_sb[:, WC_O:WC_O + 128]
        wci_a = wb_sb[:, WCI_O:WCI_O + E]
        bv_a = cin_sb[:, 0:1]
        b2_a = cin_sb[:, 1:2]

        block = EN(nc.Block())

        @block.gpsimd
        def _(gp):
            gp.memset(wmm_sb[:], 0.0).then_inc(sem_pool, 1)

        @block.sync
        def _(sp):
            sp.dma_start(xtb_sb[:, 0:512], xtb[:, 0:512]).then_inc(sem_xta, 16)
            sp.dma_start(xtb_sb[:, 512:N], xtb[:, 512:N]).then_inc(sem_xtb, 16)
            sp.dma_start(xtb_sb[:, N:XTB_W], xtb[:, N:XTB_W]).then_inc(sem_xt2, 16)
            for s_ in range(NSLAB):
                sp.dma_start(
                    adj_sb[s_][:], adj[s_ * 128:(s_ + 1) * 128, :]
                ).then_inc(sem_adj, 16)
            for s_, q_ in ((0, 0), (0, 1), (0, 2), (0, 3), (1, 0), (1, 2)):
                sp.wait_ge(sem_vec, 3 + 4 * s_ + q_)  # hj h1 + hb + mask quarters
                sp.dma_start(
                    out_d[s_ * 128:(s_ + 1) * 128, q_ * 256:(q_ + 1) * 256],
                    o_sb[s_][:, q_ * 256:(q_ + 1) * 256],
                ).then_inc(sem_out, 16)

        @block.tensor
        def _(pe):
            pe.wait_ge(sem_pool, 1)
            nc.tensor.matmul(acc_ps[0][0:2, 0:2], wmm_sb[:, 0:2], wmm_sb[:, 2:4])
            pe.wait_ge(sem_xta, 16)
            pe.wait_ge(sem_wb, 16)
            nc.tensor.matmul(hjp[0][:], wc_a, xt_a[:, 0:512]).then_inc(sem_pe, 1)
            pe.wait_ge(sem_xtb, 16)
            nc.tensor.matmul(hjp[1][:], wc_a, xt_a[:, 512:1024]).then_inc(sem_pe, 1)
            pe.wait_ge(sem_xt2, 16)
            xt2_v = xtb_sb[:, N:XTB_W].rearrange("p (g k) -> p g k", k=4)
            for k in range(4):
                nc.tensor.matmul(
                    hb_ps[32 * k:32 * (k + 1), 0:NG], wci_a, xt2_v[:, :, k],
                    tile_position=(0, 32 * k),
                ).then_inc(sem_pe, 1)
            # main: 2 slabs x 32 groups, accumulate in psum
            for g in range(NG):
                sl, gl = divmod(g, GPS)
                if g in SCA_GG:
                    pe.wait_ge(sem_sR, scnt[g])
                else:
                    pe.wait_ge(sem_vR, vcnt[g])
                w_ap = wb_sb[:, SGN_O + 124 - 4 * gl: SGN_O + 252 - 4 * gl]
                r = r_sb[g % RBUF]
                for h in range(2):
                    nc.tensor.matmul(
                        acc_ps[2 * sl + h][:], w_ap, r[:, h * 512:(h + 1) * 512],
                        start=(gl == 0), stop=(gl == GPS - 1),
                        skip_group_check=True,
                    ).then_inc(sem_pe, 1)

        def r_war_wait(eng, g):
            # overwrite slot g%RBUF: its previous tenant g-RBUF must be
            # fully consumed (2 matmuls each, after the prep matmuls)
            if g >= RBUF:
                eng.wait_ge(sem_pe, N_PREP_MM + 2 * (g - RBUF + 1))

        @block.scalar
        def _(sc):
            sc.dma_start(wb_sb[:], wb[:]).then_inc(sem_wb, 16)
            sc.dma_start(cin_sb[:], cin[:]).then_inc(sem_cin, 16)
            nc.scalar.memzero(warm_sb[:]).then_inc(sem_warm, 1)
            sc.wait_ge(sem_warm, 1)
            nc.scalar.activation(warm_sb[:], warm_sb[:], AF.Sigmoid)
            sc.wait_ge(sem_cin, 16)
            sc.wait_ge(sem_pe, 1)
            nc.scalar.activation(
                hj_sb[:, 0:512], hjp[0][:], AF.Copy,
            ).then_inc(sem_act, 1)

            def sigmoid(sl):
                sc.wait_ge(sem_pe, N_PREP_MM + 2 * GPS * (sl + 1))
                for q in range(4):
                    nc.scalar.activation(
                        o_sb[sl][:, q * 256:(q + 1) * 256],
                        acc_ps[2 * sl + q // 2][:, (q % 2) * 256:(q % 2) * 256 + 256],
                        AF.Sigmoid, bias=b2_a, scale=1.0,
                    ).then_inc(sem_act, 1)

            done0 = [False]
            first = True
            for g in SCA_GG:
                if first:
                    sc.wait_ge(sem_vec, 2)  # hj h1 + hb_sb ready
                    sc.wait_ge(sem_act, 1)  # own hj h0 write committed
                    first = False
                r_war_wait(sc, g)
                nc.scalar.activation(
                    r_sb[g % RBUF][:], hj_sb[:], AF.Relu,
                    bias=hb_sb[:, g:g + 1], scale=1.0,
                ).then_inc(sem_sR, 1)
                if g >= 42 and not done0[0]:
                    done0[0] = True
                    sigmoid(0)
            sigmoid(1)
            for q_ in (1, 3):
                sc.wait_ge(sem_vec, 3 + 4 + q_)
                sc.dma_start(
                    out_d[128:256, q_ * 256:(q_ + 1) * 256],
                    o_sb[1][:, q_ * 256:(q_ + 1) * 256],
                ).then_inc(sem_out, 16)

        @block.vector
        def _(ve):
            ve.wait_ge(sem_pe, 2)
            nc.vector.tensor_copy(
                hj_sb[:, 512:1024], hjp[1][:],
            ).then_inc(sem_vec, 1)
            ve.wait_ge(sem_cin, 16)
            ve.wait_ge(sem_pe, N_PREP_MM)
            nc.vector.tensor_scalar(
                hb_sb[:], hb_ps[:, 0:NG], bv_a, None, ALU.add,
            ).then_inc(sem_vec, 1)

            def mask(sl):
                ve.wait_ge(sem_adj, 32)
                for q in range(4):
                    ve.wait_ge(sem_act, 2 + 4 * sl + q)  # sigmoid quarter done
                    nc.vector.tensor_mul(
                        o_sb[sl][:, q * 256:(q + 1) * 256],
                        o_sb[sl][:, q * 256:(q + 1) * 256],
                        adj_sb[sl][:, q * 256:(q + 1) * 256],
                    ).then_inc(sem_vec, 1)

            done0 = [False]
            first = True
            for g in VEC_GG:
                if first:
                    ve.wait_ge(sem_act, 1)  # hj h0 ready
                    ve.wait_ge(sem_vec, 2)  # own hj h1 + hb writes committed
                    first = False
                r_war_wait(ve, g)
                nc.vector.tensor_scalar(
                    r_sb[g % RBUF][:], hj_sb[:], hb_sb[:, g:g + 1], 0.0,
                    ALU.add, ALU.max,
                ).then_inc(sem_vR, 1)
                if g >= 44 and not done0[0]:
                    done0[0] = True
                    mask(0)
            mask(1)

    return nc


def _host_prep(node_features, adjacency_matrix, Wt, bt, W1, b1, W2, b2):
    """Build per-core input maps (numpy only: resharding + weight algebra)."""
    import ml_dtypes

    f = np.float32
    bf = ml_dtypes.bfloat16
    W2v = np.asarray(W2, f)[0]                 # [E]
    aW2 = np.abs(W2v)
    sW2 = np.sign(W2v).astype(f)
    W1 = np.asarray(W1, f)
    W1i, W1j = W1[:, :H], W1[:, H:]            # [E, H]
    Wt = np.asarray(Wt, f)                     # [o, h]
    bt = np.asarray(bt, f)
    b1 = np.asarray(b1, f)
    b2c = float(np.asarray(b2, f).reshape(-1)[0])

    WtW1j = W1j @ Wt                           # [E, H]: sum_o W1j[e,o] Wt[o,h]
    WtW1i = W1i @ Wt
    allb = b1 + W1j @ bt + W1i @ bt            # [E] constant part of relu arg

    wb = np.zeros((128, WB_W), bf)
    for k in range(4):
        wb[32 * k:32 * (k + 1), SGN_O + 124 + k] = sW2.astype(bf)
    wb[:, WC_O:WC_O + 128] = np.tile((aW2[:, None] * WtW1j).T, (1, 4)).astype(bf)
    wb[:, WCI_O:WCI_O + E] = (aW2[:, None] * WtW1i).T.astype(bf)

    cin = np.empty((128, CIN_W), f)
    cin[:, 0] = np.tile(aW2 * allb, 4)
    cin[:, 1] = b2c

    in_maps = []
    adjacency_matrix = np.asarray(adjacency_matrix, f)
    node_features = np.asarray(node_features, f)
    for c in range(NCORES):
        b, i0 = c // (NCORES // B), ROWS_PER_CORE * (c % (NCORES // B))
        x = node_features[b]
        xtb = np.empty((128, XTB_W), bf)
        xtb[:, 0:N] = x.T.astype(bf)
        xtb[:, N:] = x[i0:i0 + ROWS_PER_CORE].T.astype(bf)
        in_maps.append(dict(
            xtb=xtb, wb=wb, cin=cin,
            adj=np.ascontiguousarray(adjacency_matrix[i0:i0 + ROWS_PER_CORE]),
        ))
    return in_maps


def run(inputs, trace=False):
    from concourse.bass_utils import run_bass_kernel_spmd

    if "prog" not in _CACHE:
        _CACHE["prog"] = _build_program()
    nc = _CACHE["prog"]
    in_maps = _host_prep(**inputs)
    res = run_bass_kernel_spmd(nc, in_maps, list(range(NCORES)), trace=trace)
    out = np.empty((B, N, N), np.float32)
    for c in range(NCORES):
        b, i0 = c // (NCORES // B), ROWS_PER_CORE * (c % (NCORES // B))
        out[b, i0:i0 + ROWS_PER_CORE] = res.results[c]["out"]
    return out, res


def kernel(**inputs):
    out, _ = run(inputs, trace=False)
    return out

